# revision 5
# baseline (speedup 1.0000x reference)
"""Trainium2 Bass kernel for nn_BasisLinear (B=65536, Cin=64, Cout=64, Rin=Rout=4, R=16).

The module computes, per batch row b:
    out[b, O, p] = sum_{I,q} W[O,p,I,q] * x[b,I,q] + bias[O,p]
with W = einsum('rpq,rOI->OpIq', basis, coeffs) a tiny [256, 256] matrix and
bias = einsum('rp,rO->Op') a [256] vector — i.e. a plain 256->256 linear layer
over the flattened feature dim, batch 65536.

Strategy (data-parallel over batch across 8 cores, per the sharding hint):
  * Host folds basis/coeffs into W^T [256(f_in), 256(f_out)] and bias [128, 2].
  * Host shards x into 8 x [8192, 256] and TRANSPOSES each shard to
    xT [256, 8192].  With f_in on partitions every device DMA is fully
    contiguous (8 KiB per partition per chunk) — fp32 transposes on-chip are
    the one expensive thing on trn2, so they happen here, in the shard step.
  * Device: out_psum[f_out, b] = sum_k  wT[k_tile, f_out_tile]^T @ xT[k_tile, b]
    (K = 256 split in 2, f_out = 256 split in 2, moving N = 512 batch columns).
    Bias is a per-partition scalar added by the scalar engine during
    PSUM->SBUF evacuation.  Stores go out as outT [256, 8192].
  * Host transposes shards back and stacks to [65536, 64, 4].

Matmul modes:
  f32    — native fp32 matmul, exact, 4 cycles/column on the PE.
  bf16x3 — host splits x and W into bf16 hi+lo; device computes
           xh*wh + xh*wl + xl*wh (3 bf16 matmuls, 1 cycle/column each,
           fp32 PSUM accumulate).  ~1e-5 relative error, 25% fewer PE
           cycles than f32; same DMA bytes (hi+lo bf16 == 4 B/elem).
  f32r   — tf32-like 11-mantissa-bit hw mode, 1 cycle/column; inputs
           pre-rounded on host (fp32r rounding is idempotent).  ~3e-4 err.
"""

import numpy as np
import ml_dtypes

import concourse.bacc as bacc
import concourse.mybir as mybir
import concourse.tile as tile
from concourse import bass_utils

N_CORES = 8
B = 65536
F = 256            # Cin*Rin == Cout*Rout
B_CORE = B // N_CORES

CHUNK = 2048       # batch columns per DMA chunk
SUB = 512          # moving free dim per matmul (fp32 max)
MODE = "bf16x3"


def build_program(mode=MODE, chunk=CHUNK, b_core=B_CORE):
    """Build + compile the SPMD Bass program (same NEFF on all 8 cores)."""
    n_chunks = b_core // chunk
    subs = chunk // SUB
    f32 = mybir.dt.float32
    bf16 = mybir.dt.bfloat16
    f32r = mybir.dt.float32r

    nc = bacc.Bacc("TRN2", target_bir_lowering=False, debug=False,
                   num_devices=N_CORES)

    if mode == "bf16x3":
        # hi/lo bf16 planes of xT, stored stacked: [2(k-half)*128, b_core] each
        xh = nc.dram_tensor("xh", (F, b_core), bf16, kind="ExternalInput")
        xl = nc.dram_tensor("xl", (F, b_core), bf16, kind="ExternalInput")
        wh = nc.dram_tensor("wh", (F, F), bf16, kind="ExternalInput")
        wl = nc.dram_tensor("wl", (F, F), bf16, kind="ExternalInput")
        x_drams = (xh, xl)
        w_drams = (wh, wl)
        mm_dt = bf16
    else:
        mm_dt = f32 if mode == "f32" else f32r
        xT = nc.dram_tensor("xT", (F, b_core), mm_dt, kind="ExternalInput")
        wT = nc.dram_tensor("wT", (F, F), mm_dt, kind="ExternalInput")
        x_drams = (xT,)
        w_drams = (wT,)
    bias = nc.dram_tensor("bias", (128, 2), f32, kind="ExternalInput")
    outT = nc.dram_tensor("outT", (F, b_core), f32, kind="ExternalOutput")

    with tile.TileContext(nc) as tc:
        with (
            tc.tile_pool(name="consts", bufs=1) as consts,
            tc.tile_pool(name="xbuf", bufs=3) as xbuf,
            tc.tile_pool(name="obuf", bufs=3) as obuf,
            tc.tile_pool(name="psum", bufs=8, space="PSUM") as psum,
        ):
            # weights: per plane, w_sb[:, ki*F:(ki+1)*F] = wX[ki*128:(ki+1)*128, :]
            w_sbs = []
            for wi, wd in enumerate(w_drams):
                w_sb = consts.tile([128, 2 * F], mm_dt, name=f"w_sb{wi}")
                for ki in range(2):
                    nc.sync.dma_start(out=w_sb[:, ki * F:(ki + 1) * F],
                                      in_=wd.ap()[ki * 128:(ki + 1) * 128, :])
                w_sbs.append(w_sb)
            bias_sb = consts.tile([128, 2], f32)
            nc.sync.dma_start(out=bias_sb[:], in_=bias.ap())

            for c in range(n_chunks):
                csl = slice(c * chunk, (c + 1) * chunk)
                # x tiles: x_sbs[plane][ki]
                x_sbs = []
                for xi, xd in enumerate(x_drams):
                    xk0 = xbuf.tile([128, chunk], mm_dt, tag=f"x{xi}k0",
                                    name=f"x{xi}k0_{c}")
                    xk1 = xbuf.tile([128, chunk], mm_dt, tag=f"x{xi}k1",
                                    name=f"x{xi}k1_{c}")
                    nc.sync.dma_start(out=xk0[:], in_=xd.ap()[0:128, csl])
                    nc.sync.dma_start(out=xk1[:], in_=xd.ap()[128:256, csl])
                    x_sbs.append((xk0, xk1))
                o0 = obuf.tile([128, chunk], f32, tag="o0", name=f"o0_{c}")
                o1 = obuf.tile([128, chunk], f32, tag="o1", name=f"o1_{c}")
                outs = (o0, o1)

                # (x_plane, w_plane) matmul terms accumulated into psum
                if mode == "bf16x3":
                    terms = ((0, 0), (0, 1), (1, 0))   # xh*wh + xh*wl + xl*wh
                else:
                    terms = ((0, 0),)

                # sub-pairs of 2x512 columns share each weight load
                for sp in range(subs // 2):
                    for mi in range(2):
                        ps = [
                            psum.tile([128, SUB], f32, tag="ps",
                                      name=f"ps_{c}_{sp}_{mi}_{s}")
                            for s in range(2)
                        ]
                        first, last = terms[0], terms[-1]
                        for ki in range(2):
                            for t in terms:
                                xp, wp = t
                                w_ap = w_sbs[wp][:, ki * F + mi * 128:
                                                 ki * F + (mi + 1) * 128]
                                for s in range(2):
                                    ssl = slice((2 * sp + s) * SUB,
                                                (2 * sp + s + 1) * SUB)
                                    nc.tensor.matmul(
                                        ps[s][:], w_ap,
                                        x_sbs[xp][ki][:, ssl],
                                        start=(ki == 0 and t == first),
                                        stop=(ki == 1 and t == last))
                        for s in range(2):
                            ssl = slice((2 * sp + s) * SUB,
                                        (2 * sp + s + 1) * SUB)
                            nc.scalar.add(out=outs[mi][:, ssl], in_=ps[s][:],
                                          add=bias_sb[:, mi:mi + 1])

                nc.scalar.dma_start(out=outT.ap()[0:128, csl], in_=o0[:])
                nc.scalar.dma_start(out=outT.ap()[128:256, csl], in_=o1[:])

    nc.compile()
    return nc


def round_fp32r(a):
    """Round-to-nearest-even to 11 mantissa bits (matches hw fp32r)."""
    u = a.view(np.uint32)
    keep = np.uint32(0xFFFFF000)
    lsb = (u >> np.uint32(12)) & np.uint32(1)
    r = (u + np.uint32(0x7FF) + lsb) & keep
    return r.view(np.float32)


def split_bf16(a):
    """a (fp32) -> (hi, lo) bf16 with hi + lo ≈ a to ~16 mantissa bits."""
    hi = a.astype(ml_dtypes.bfloat16)
    lo = (a - hi.astype(np.float32)).astype(ml_dtypes.bfloat16)
    return hi, lo


def host_prepack(basis, coeffs, basis_bias, coeffs_bias):
    """Fold the basis factorization into wT [256,256] and bias [128,2]."""
    b_sq = np.asarray(basis, np.float32)[:, 0, :, 0, :]     # [R, p, q]
    c_sq = np.asarray(coeffs, np.float32)[:, :, 0, :, 0]    # [R, O, I]
    # W[O,p,I,q] -> flat [f_out, f_in]
    W = np.einsum("rpq,rOI->OpIq", b_sq, c_sq)
    w_flat = np.ascontiguousarray(W.reshape(F, F))
    wT = np.ascontiguousarray(w_flat.T)                     # [f_in, f_out]
    bb = np.asarray(basis_bias, np.float32)[:, 0, :]        # [Rb, p]
    cb = np.asarray(coeffs_bias, np.float32)[:, :, 0]       # [Rb, O]
    bias_vec = np.einsum("rp,rO->Op", bb, cb).reshape(F)    # [f_out]
    bias_mat = np.ascontiguousarray(bias_vec.reshape(2, 128).T)  # [128, 2]
    return wT, bias_mat


def make_in_maps(x, basis, coeffs, basis_bias, coeffs_bias, mode=MODE):
    wT, bias_mat = host_prepack(basis, coeffs, basis_bias, coeffs_bias)
    x2 = np.ascontiguousarray(np.asarray(x, np.float32)).reshape(B, F)
    if mode == "f32r":
        wT = round_fp32r(wT)
        x2 = round_fp32r(x2)
    in_maps = []
    if mode == "bf16x3":
        wh, wl = split_bf16(wT)
        for c in range(N_CORES):
            shard_t = np.ascontiguousarray(
                x2[c * B_CORE:(c + 1) * B_CORE].T)          # [F, B_CORE]
            xh, xl = split_bf16(shard_t)
            in_maps.append({"xh": np.ascontiguousarray(xh),
                            "xl": np.ascontiguousarray(xl),
                            "wh": wh, "wl": wl, "bias": bias_mat})
    else:
        for c in range(N_CORES):
            xT_c = np.ascontiguousarray(x2[c * B_CORE:(c + 1) * B_CORE].T)
            in_maps.append({"xT": xT_c, "wT": wT, "bias": bias_mat})
    return in_maps


def assemble_out(results):
    out = np.empty((B, F), np.float32)
    for c in range(N_CORES):
        out[c * B_CORE:(c + 1) * B_CORE] = results[c]["outT"].T
    return out.reshape(B, 64, 4)


_PROGRAM = None


def kernel(x, basis, coeffs, basis_bias, coeffs_bias):
    global _PROGRAM
    if _PROGRAM is None:
        _PROGRAM = build_program()
    in_maps = make_in_maps(x, basis, coeffs, basis_bias, coeffs_bias)
    res = bass_utils.run_bass_kernel_spmd(
        _PROGRAM, in_maps, core_ids=list(range(N_CORES)))
    return assemble_out(res.results)


# revision 8
# speedup vs baseline: 1.1191x; 1.1191x over previous
"""Trainium2 Bass kernel for nn_BasisLinear (B=65536, Cin=64, Cout=64, Rin=Rout=4, R=16).

The module computes, per batch row b:
    out[b, O, p] = sum_{I,q} W[O,p,I,q] * x[b,I,q] + bias[O,p]
with W = einsum('rpq,rOI->OpIq', basis, coeffs) a tiny [256, 256] matrix and
bias = einsum('rp,rO->Op') a [256] vector — i.e. a plain 256->256 linear layer
over the flattened feature dim, batch 65536.

Strategy (data-parallel over batch across 8 cores, per the sharding hint):
  * Host folds basis/coeffs into W^T [256(f_in), 256(f_out)] and bias [128, 2].
  * Host shards x into 8 x [8192, 256] and TRANSPOSES each shard to
    xT [256, 8192].  With f_in on partitions every device DMA is fully
    contiguous (8 KiB per partition per chunk) — fp32 transposes on-chip are
    the one expensive thing on trn2, so they happen here, in the shard step.
  * Device: out_psum[f_out, b] = sum_k  wT[k_tile, f_out_tile]^T @ xT[k_tile, b]
    (K = 256 split in 2, f_out = 256 split in 2, moving N = 512 batch columns).
    Bias is a per-partition scalar added by the scalar engine during
    PSUM->SBUF evacuation.  Stores go out as outT [256, 8192].
  * Host transposes shards back and stacks to [65536, 64, 4].

Matmul modes:
  f32    — native fp32 matmul, exact, 4 cycles/column on the PE.
  bf16x3 — host splits x and W into bf16 hi+lo; device computes
           xh*wh + xh*wl + xl*wh (3 bf16 matmuls, 1 cycle/column each,
           fp32 PSUM accumulate).  ~1e-5 relative error, 25% fewer PE
           cycles than f32; same DMA bytes (hi+lo bf16 == 4 B/elem).
  f32r   — tf32-like 11-mantissa-bit hw mode, 1 cycle/column; inputs
           pre-rounded on host (fp32r rounding is idempotent).  ~3e-4 err.
"""

import numpy as np
import ml_dtypes

import concourse.bacc as bacc
import concourse.mybir as mybir
import concourse.tile as tile
from concourse import bass_utils

N_CORES = 8
B = 65536
F = 256            # Cin*Rin == Cout*Rout
B_CORE = B // N_CORES

CHUNK = 1024       # batch columns per DMA chunk
SUB = 512          # moving free dim per matmul (fp32 max)
MODE = "bf16x3"


def build_program(mode=MODE, chunk=CHUNK, b_core=B_CORE):
    """Build + compile the SPMD Bass program (same NEFF on all 8 cores)."""
    n_chunks = b_core // chunk
    subs = chunk // SUB
    f32 = mybir.dt.float32
    bf16 = mybir.dt.bfloat16
    f32r = mybir.dt.float32r

    nc = bacc.Bacc("TRN2", target_bir_lowering=False, debug=False,
                   num_devices=N_CORES)

    if mode == "bf16x3":
        # hi/lo bf16 planes of xT, stored stacked: [2(k-half)*128, b_core] each
        xh = nc.dram_tensor("xh", (F, b_core), bf16, kind="ExternalInput")
        xl = nc.dram_tensor("xl", (F, b_core), bf16, kind="ExternalInput")
        wh = nc.dram_tensor("wh", (F, F), bf16, kind="ExternalInput")
        wl = nc.dram_tensor("wl", (F, F), bf16, kind="ExternalInput")
        x_drams = (xh, xl)
        w_drams = (wh, wl)
        mm_dt = bf16
    else:
        mm_dt = f32 if mode == "f32" else f32r
        xT = nc.dram_tensor("xT", (F, b_core), mm_dt, kind="ExternalInput")
        wT = nc.dram_tensor("wT", (F, F), mm_dt, kind="ExternalInput")
        x_drams = (xT,)
        w_drams = (wT,)
    bias = nc.dram_tensor("bias", (128, 2), f32, kind="ExternalInput")
    outT = nc.dram_tensor("outT", (F, b_core), f32, kind="ExternalOutput")

    with tile.TileContext(nc) as tc:
        with (
            tc.tile_pool(name="consts", bufs=1) as consts,
            tc.tile_pool(name="xbuf", bufs=4) as xbuf,
            tc.tile_pool(name="obuf", bufs=4) as obuf,
            tc.tile_pool(name="psum", bufs=8, space="PSUM") as psum,
        ):
            # weights: per plane, w_sb[:, ki*F:(ki+1)*F] = wX[ki*128:(ki+1)*128, :]
            w_sbs = []
            for wi, wd in enumerate(w_drams):
                w_sb = consts.tile([128, 2 * F], mm_dt, name=f"w_sb{wi}")
                for ki in range(2):
                    nc.sync.dma_start(out=w_sb[:, ki * F:(ki + 1) * F],
                                      in_=wd.ap()[ki * 128:(ki + 1) * 128, :])
                w_sbs.append(w_sb)
            bias_sb = consts.tile([128, 2], f32)
            nc.sync.dma_start(out=bias_sb[:], in_=bias.ap())

            for c in range(n_chunks):
                csl = slice(c * chunk, (c + 1) * chunk)
                # x tiles: x_sbs[plane][ki]
                x_sbs = []
                for xi, xd in enumerate(x_drams):
                    xk0 = xbuf.tile([128, chunk], mm_dt, tag=f"x{xi}k0",
                                    name=f"x{xi}k0_{c}")
                    xk1 = xbuf.tile([128, chunk], mm_dt, tag=f"x{xi}k1",
                                    name=f"x{xi}k1_{c}")
                    nc.sync.dma_start(out=xk0[:], in_=xd.ap()[0:128, csl])
                    nc.sync.dma_start(out=xk1[:], in_=xd.ap()[128:256, csl])
                    x_sbs.append((xk0, xk1))
                o0 = obuf.tile([128, chunk], f32, tag="o0", name=f"o0_{c}")
                o1 = obuf.tile([128, chunk], f32, tag="o1", name=f"o1_{c}")
                outs = (o0, o1)

                # (x_plane, w_plane) matmul terms accumulated into psum
                if mode == "bf16x3":
                    terms = ((0, 0), (0, 1), (1, 0))   # xh*wh + xh*wl + xl*wh
                else:
                    terms = ((0, 0),)

                # sub-pairs of 2x512 columns share each weight load
                for sp in range(subs // 2):
                    for mi in range(2):
                        ps = [
                            psum.tile([128, SUB], f32, tag="ps",
                                      name=f"ps_{c}_{sp}_{mi}_{s}")
                            for s in range(2)
                        ]
                        first, last = terms[0], terms[-1]
                        for ki in range(2):
                            for t in terms:
                                xp, wp = t
                                w_ap = w_sbs[wp][:, ki * F + mi * 128:
                                                 ki * F + (mi + 1) * 128]
                                for s in range(2):
                                    ssl = slice((2 * sp + s) * SUB,
                                                (2 * sp + s + 1) * SUB)
                                    nc.tensor.matmul(
                                        ps[s][:], w_ap,
                                        x_sbs[xp][ki][:, ssl],
                                        start=(ki == 0 and t == first),
                                        stop=(ki == 1 and t == last))
                        for s in range(2):
                            ssl = slice((2 * sp + s) * SUB,
                                        (2 * sp + s + 1) * SUB)
                            # split PSUM evacuation across ACT and DVE
                            if s == 0:
                                nc.scalar.add(out=outs[mi][:, ssl],
                                              in_=ps[s][:],
                                              add=bias_sb[:, mi:mi + 1])
                            else:
                                nc.vector.tensor_scalar_add(
                                    out=outs[mi][:, ssl], in0=ps[s][:],
                                    scalar1=bias_sb[:, mi:mi + 1])

                nc.scalar.dma_start(out=outT.ap()[0:128, csl], in_=o0[:])
                nc.scalar.dma_start(out=outT.ap()[128:256, csl], in_=o1[:])

    nc.compile()
    return nc


def round_fp32r(a):
    """Round-to-nearest-even to 11 mantissa bits (matches hw fp32r)."""
    u = a.view(np.uint32)
    keep = np.uint32(0xFFFFF000)
    lsb = (u >> np.uint32(12)) & np.uint32(1)
    r = (u + np.uint32(0x7FF) + lsb) & keep
    return r.view(np.float32)


def split_bf16(a):
    """a (fp32) -> (hi, lo) bf16 with hi + lo ≈ a to ~16 mantissa bits."""
    hi = a.astype(ml_dtypes.bfloat16)
    lo = (a - hi.astype(np.float32)).astype(ml_dtypes.bfloat16)
    return hi, lo


def host_prepack(basis, coeffs, basis_bias, coeffs_bias):
    """Fold the basis factorization into wT [256,256] and bias [128,2]."""
    b_sq = np.asarray(basis, np.float32)[:, 0, :, 0, :]     # [R, p, q]
    c_sq = np.asarray(coeffs, np.float32)[:, :, 0, :, 0]    # [R, O, I]
    # W[O,p,I,q] -> flat [f_out, f_in]
    W = np.einsum("rpq,rOI->OpIq", b_sq, c_sq)
    w_flat = np.ascontiguousarray(W.reshape(F, F))
    wT = np.ascontiguousarray(w_flat.T)                     # [f_in, f_out]
    bb = np.asarray(basis_bias, np.float32)[:, 0, :]        # [Rb, p]
    cb = np.asarray(coeffs_bias, np.float32)[:, :, 0]       # [Rb, O]
    bias_vec = np.einsum("rp,rO->Op", bb, cb).reshape(F)    # [f_out]
    bias_mat = np.ascontiguousarray(bias_vec.reshape(2, 128).T)  # [128, 2]
    return wT, bias_mat


def make_in_maps(x, basis, coeffs, basis_bias, coeffs_bias, mode=MODE):
    wT, bias_mat = host_prepack(basis, coeffs, basis_bias, coeffs_bias)
    x2 = np.ascontiguousarray(np.asarray(x, np.float32)).reshape(B, F)
    if mode == "f32r":
        wT = round_fp32r(wT)
        x2 = round_fp32r(x2)
    in_maps = []
    if mode == "bf16x3":
        wh, wl = split_bf16(wT)
        for c in range(N_CORES):
            shard_t = np.ascontiguousarray(
                x2[c * B_CORE:(c + 1) * B_CORE].T)          # [F, B_CORE]
            xh, xl = split_bf16(shard_t)
            in_maps.append({"xh": np.ascontiguousarray(xh),
                            "xl": np.ascontiguousarray(xl),
                            "wh": wh, "wl": wl, "bias": bias_mat})
    else:
        for c in range(N_CORES):
            xT_c = np.ascontiguousarray(x2[c * B_CORE:(c + 1) * B_CORE].T)
            in_maps.append({"xT": xT_c, "wT": wT, "bias": bias_mat})
    return in_maps


def assemble_out(results):
    out = np.empty((B, F), np.float32)
    for c in range(N_CORES):
        out[c * B_CORE:(c + 1) * B_CORE] = results[c]["outT"].T
    return out.reshape(B, 64, 4)


_PROGRAM = None


def kernel(x, basis, coeffs, basis_bias, coeffs_bias):
    global _PROGRAM
    if _PROGRAM is None:
        _PROGRAM = build_program()
    in_maps = make_in_maps(x, basis, coeffs, basis_bias, coeffs_bias)
    res = bass_utils.run_bass_kernel_spmd(
        _PROGRAM, in_maps, core_ids=list(range(N_CORES)))
    return assemble_out(res.results)


# revision 11
# speedup vs baseline: 1.1253x; 1.0056x over previous
"""Trainium2 Bass kernel for nn_BasisLinear (B=65536, Cin=64, Cout=64, Rin=Rout=4, R=16).

The module computes, per batch row b:
    out[b, O, p] = sum_{I,q} W[O,p,I,q] * x[b,I,q] + bias[O,p]
with W = einsum('rpq,rOI->OpIq', basis, coeffs) a tiny [256, 256] matrix and
bias = einsum('rp,rO->Op') a [256] vector — i.e. a plain 256->256 linear layer
over the flattened feature dim, batch 65536.

Strategy (data-parallel over batch across 8 cores, per the sharding hint):
  * Host folds basis/coeffs into W^T [256(f_in), 256(f_out)] and bias [128, 2].
  * Host shards x into 8 x [8192, 256] and TRANSPOSES each shard to
    xT [256, 8192].  With f_in on partitions every device DMA is fully
    contiguous (8 KiB per partition per chunk) — fp32 transposes on-chip are
    the one expensive thing on trn2, so they happen here, in the shard step.
  * Device: out_psum[f_out, b] = sum_k  wT[k_tile, f_out_tile]^T @ xT[k_tile, b]
    (K = 256 split in 2, f_out = 256 split in 2, moving N = 512 batch columns).
    Bias is a per-partition scalar added by the scalar engine during
    PSUM->SBUF evacuation.  Stores go out as outT [256, 8192].
  * Host transposes shards back and stacks to [65536, 64, 4].

Matmul modes:
  f32    — native fp32 matmul, exact, 4 cycles/column on the PE.
  bf16x3 — host splits x and W into bf16 hi+lo; device computes
           xh*wh + xh*wl + xl*wh (3 bf16 matmuls, 1 cycle/column each,
           fp32 PSUM accumulate).  ~1e-5 relative error, 25% fewer PE
           cycles than f32; same DMA bytes (hi+lo bf16 == 4 B/elem).
  f32r   — tf32-like 11-mantissa-bit hw mode, 1 cycle/column; inputs
           pre-rounded on host (fp32r rounding is idempotent).  ~3e-4 err.
"""

import numpy as np
import ml_dtypes

import concourse.bacc as bacc
import concourse.mybir as mybir
import concourse.tile as tile
from concourse import bass_utils

N_CORES = 8
B = 65536
F = 256            # Cin*Rin == Cout*Rout
B_CORE = B // N_CORES

CHUNK = 1024       # batch columns per DMA chunk
SUB = 512          # moving free dim per matmul (fp32 max)
MODE = "bf16x3"


def build_program(mode=MODE, chunk=CHUNK, b_core=B_CORE):
    """Build + compile the SPMD Bass program (same NEFF on all 8 cores)."""
    n_chunks = b_core // chunk
    subs = chunk // SUB
    f32 = mybir.dt.float32
    bf16 = mybir.dt.bfloat16
    f32r = mybir.dt.float32r

    nc = bacc.Bacc("TRN2", target_bir_lowering=False, debug=False,
                   num_devices=N_CORES)

    if mode == "bf16x3":
        # hi/lo bf16 planes of xT, stored stacked: [2(k-half)*128, b_core] each
        xh = nc.dram_tensor("xh", (F, b_core), bf16, kind="ExternalInput")
        xl = nc.dram_tensor("xl", (F, b_core), bf16, kind="ExternalInput")
        x_drams = (xh, xl)
        n_w = 2
        mm_dt = bf16
        welems = 1   # dram/sbuf elements per weight value
    else:
        mm_dt = f32 if mode == "f32" else f32r
        xT = nc.dram_tensor("xT", (F, b_core), mm_dt, kind="ExternalInput")
        x_drams = (xT,)
        n_w = 1
        welems = 2   # view fp32 weights as 2 bf16 columns for the packed load
    # All constants in one DMA: n_w weight planes, each [128, 2*F] as
    # w[ki*128+p, f] -> wpack[p, plane*2F + ki*F + f], then 4 bf16 columns
    # holding the fp32 bias [128, 2].
    wpack = nc.dram_tensor("wpack", (128, n_w * 2 * F * welems + 4), bf16,
                           kind="ExternalInput")
    outT = nc.dram_tensor("outT", (F, b_core), f32, kind="ExternalOutput")

    with tile.TileContext(nc) as tc:
        with (
            tc.tile_pool(name="consts", bufs=1) as consts,
            tc.tile_pool(name="xbuf", bufs=4) as xbuf,
            tc.tile_pool(name="obuf", bufs=4) as obuf,
            tc.tile_pool(name="psum", bufs=8, space="PSUM") as psum,
        ):
            # one DMA for all constants (weights + bias)
            wpack_sb = consts.tile([128, n_w * 2 * F * welems + 4], bf16)
            nc.sync.dma_start(out=wpack_sb[:], in_=wpack.ap())
            # w_sbs[plane][:, ki*F + f] views (dtype mm_dt)
            w_sbs = [
                wpack_sb[:, wi * 2 * F * welems:
                         (wi + 1) * 2 * F * welems].bitcast(mm_dt)
                for wi in range(n_w)
            ]
            bias_sb = wpack_sb[:, n_w * 2 * F * welems:
                               n_w * 2 * F * welems + 4].bitcast(f32)

            for c in range(n_chunks):
                csl = slice(c * chunk, (c + 1) * chunk)
                # x tiles: x_sbs[plane][ki]
                x_sbs = []
                for xi, xd in enumerate(x_drams):
                    xk0 = xbuf.tile([128, chunk], mm_dt, tag=f"x{xi}k0",
                                    name=f"x{xi}k0_{c}")
                    xk1 = xbuf.tile([128, chunk], mm_dt, tag=f"x{xi}k1",
                                    name=f"x{xi}k1_{c}")
                    nc.sync.dma_start(out=xk0[:], in_=xd.ap()[0:128, csl])
                    nc.sync.dma_start(out=xk1[:], in_=xd.ap()[128:256, csl])
                    x_sbs.append((xk0, xk1))
                o0 = obuf.tile([128, chunk], f32, tag="o0", name=f"o0_{c}")
                o1 = obuf.tile([128, chunk], f32, tag="o1", name=f"o1_{c}")
                outs = (o0, o1)

                # (x_plane, w_plane) matmul terms accumulated into psum
                if mode == "bf16x3":
                    terms = ((0, 0), (0, 1), (1, 0))   # xh*wh + xh*wl + xl*wh
                else:
                    terms = ((0, 0),)

                # sub-pairs of 2x512 columns share each weight load
                for sp in range(subs // 2):
                    for mi in range(2):
                        ps = [
                            psum.tile([128, SUB], f32, tag="ps",
                                      name=f"ps_{c}_{sp}_{mi}_{s}")
                            for s in range(2)
                        ]
                        first, last = terms[0], terms[-1]
                        for ki in range(2):
                            for t in terms:
                                xp, wp = t
                                w_ap = w_sbs[wp][:, ki * F + mi * 128:
                                                 ki * F + (mi + 1) * 128]
                                for s in range(2):
                                    ssl = slice((2 * sp + s) * SUB,
                                                (2 * sp + s + 1) * SUB)
                                    nc.tensor.matmul(
                                        ps[s][:], w_ap,
                                        x_sbs[xp][ki][:, ssl],
                                        start=(ki == 0 and t == first),
                                        stop=(ki == 1 and t == last))
                        for s in range(2):
                            ssl = slice((2 * sp + s) * SUB,
                                        (2 * sp + s + 1) * SUB)
                            # split PSUM evacuation across ACT and DVE
                            if s == 0:
                                nc.scalar.add(out=outs[mi][:, ssl],
                                              in_=ps[s][:],
                                              add=bias_sb[:, mi:mi + 1])
                            else:
                                nc.vector.tensor_scalar_add(
                                    out=outs[mi][:, ssl], in0=ps[s][:],
                                    scalar1=bias_sb[:, mi:mi + 1])

                nc.scalar.dma_start(out=outT.ap()[0:128, csl], in_=o0[:])
                nc.scalar.dma_start(out=outT.ap()[128:256, csl], in_=o1[:])

    nc.compile()
    return nc


def round_fp32r(a):
    """Round-to-nearest-even to 11 mantissa bits (matches hw fp32r)."""
    u = a.view(np.uint32)
    keep = np.uint32(0xFFFFF000)
    lsb = (u >> np.uint32(12)) & np.uint32(1)
    r = (u + np.uint32(0x7FF) + lsb) & keep
    return r.view(np.float32)


def split_bf16(a):
    """a (fp32) -> (hi, lo) bf16 with hi + lo ≈ a to ~16 mantissa bits."""
    hi = a.astype(ml_dtypes.bfloat16)
    lo = (a - hi.astype(np.float32)).astype(ml_dtypes.bfloat16)
    return hi, lo


def host_prepack(basis, coeffs, basis_bias, coeffs_bias):
    """Fold the basis factorization into wT [256,256] and bias [128,2]."""
    b_sq = np.asarray(basis, np.float32)[:, 0, :, 0, :]     # [R, p, q]
    c_sq = np.asarray(coeffs, np.float32)[:, :, 0, :, 0]    # [R, O, I]
    # W[O,p,I,q] -> flat [f_out, f_in]
    W = np.einsum("rpq,rOI->OpIq", b_sq, c_sq)
    w_flat = np.ascontiguousarray(W.reshape(F, F))
    wT = np.ascontiguousarray(w_flat.T)                     # [f_in, f_out]
    bb = np.asarray(basis_bias, np.float32)[:, 0, :]        # [Rb, p]
    cb = np.asarray(coeffs_bias, np.float32)[:, :, 0]       # [Rb, O]
    bias_vec = np.einsum("rp,rO->Op", bb, cb).reshape(F)    # [f_out]
    bias_mat = np.ascontiguousarray(bias_vec.reshape(2, 128).T)  # [128, 2]
    return wT, bias_mat


def _fold_khalf(w):
    """[256, F] -> [128, 2*F] with w[ki*128+p, f] at [p, ki*F+f]."""
    return np.ascontiguousarray(
        w.reshape(2, 128, F).transpose(1, 0, 2).reshape(128, 2 * F))


def make_in_maps(x, basis, coeffs, basis_bias, coeffs_bias, mode=MODE):
    wT, bias_mat = host_prepack(basis, coeffs, basis_bias, coeffs_bias)
    x2 = np.ascontiguousarray(np.asarray(x, np.float32)).reshape(B, F)
    if mode == "f32r":
        wT = round_fp32r(wT)
        x2 = round_fp32r(x2)

    bf = ml_dtypes.bfloat16
    parts = []
    if mode == "bf16x3":
        wh, wl = split_bf16(wT)
        parts = [_fold_khalf(wh).view(np.uint16),
                 _fold_khalf(wl).view(np.uint16)]
    else:
        parts = [_fold_khalf(wT).view(np.uint16)]
    parts.append(np.ascontiguousarray(bias_mat).view(np.uint16))
    wpack = np.ascontiguousarray(np.concatenate(parts, axis=1)).view(bf)

    in_maps = []
    if mode == "bf16x3":
        for c in range(N_CORES):
            shard_t = np.ascontiguousarray(
                x2[c * B_CORE:(c + 1) * B_CORE].T)          # [F, B_CORE]
            xh, xl = split_bf16(shard_t)
            in_maps.append({"xh": np.ascontiguousarray(xh),
                            "xl": np.ascontiguousarray(xl),
                            "wpack": wpack})
    else:
        for c in range(N_CORES):
            xT_c = np.ascontiguousarray(x2[c * B_CORE:(c + 1) * B_CORE].T)
            in_maps.append({"xT": xT_c, "wpack": wpack})
    return in_maps


def assemble_out(results):
    out = np.empty((B, F), np.float32)
    for c in range(N_CORES):
        out[c * B_CORE:(c + 1) * B_CORE] = results[c]["outT"].T
    return out.reshape(B, 64, 4)


_PROGRAM = None


def kernel(x, basis, coeffs, basis_bias, coeffs_bias):
    global _PROGRAM
    if _PROGRAM is None:
        _PROGRAM = build_program()
    in_maps = make_in_maps(x, basis, coeffs, basis_bias, coeffs_bias)
    res = bass_utils.run_bass_kernel_spmd(
        _PROGRAM, in_maps, core_ids=list(range(N_CORES)))
    return assemble_out(res.results)
